# revision 20
# baseline (speedup 1.0000x reference)
"""Trainium2 Bass kernel for nn_DescriptorNetwork (gnn_message_passing).

Self-contained: hardcodes shapes/sharding from the problem spec.

Math refactoring (validated vs reference):
  - pair = [fea[self], fea[nbr]]; pair @ W1 = fea[self] @ W1_top + fea[nbr] @ W1_bot
    -> per-site A = fea_aug @ W1a, B = fea_aug @ W1b (bias folded via ones-row),
       pair hidden h[:, c,i,j] = A[:, c,i] + B[:, c,j] built as a kt-interleaved
       outer sum on DVE (bf16 2x tensor_tensor), kt innermost so every operand
       AP has a packed last dim.
  - LeakyReLU(x) = 0.01x + 0.99 relu(x). For the gate, the 0.01-linear term
    splits into a pure-i part (cancels in the softmax ratio) and a pure-j part
    that rides the existing k=1 logit matmul row together with pow*log(w):
    row[site] = 0.01*w2g.Bg[site] + pow*logw[site]. So the gate pair path is
    add (DVE 2x) + relu (DVE tensor_scalar 4x) + matmul with 0.99*w2g.
  - gate logits are tiny: no segment_max; +1e-10 denom guard negligible.
  - msg: t = (Am+Bm) as interleaved add; LR applied after multiplying by the
    (kt-duplicated) gate e2: LR(e*h) = e*LR(h) since e>0; Act Prelu does LR.
  - msg bias b2 passes through the (sum==1) softmax: added at the end.

Layout: channels on partitions; sites/pairs along free, pair cols ordered
(c, i, j, kt) with kt (hidden-chunk) innermost. Sharding: 1024 crystals
(8192 sites) per core, weights replicated, outputs concatenated on host.
"""
import os
import numpy as np

import concourse.bacc as bacc
import concourse.tile as tile
from concourse import mybir
from concourse.bass_utils import run_bass_kernel_spmd

F32 = mybir.dt.float32
F32R = mybir.dt.float32r
BF16 = mybir.dt.bfloat16
AF = mybir.ActivationFunctionType
ALU = mybir.AluOpType

N_CRY, ELEMS, AUG = 8192, 8, 4
N = N_CRY * ELEMS
ELEM_EMB, SYM_EMB = 200, 444
FEA, HID, NG = 64, 256, 3
NCORES = 8
S = N // NCORES            # 8192 sites per core
C = N_CRY // NCORES        # 1024 crystals per core
CAUG = C // AUG            # 256 output rows per core

HB = 256                   # site half-block (= 2048 pairs)
NHB = S // HB              # 32 per layer
CHB = HB // ELEMS          # 32 crystals per half-block
PB = 512                   # pair psum block (logit chunk)
EMB_CHUNK = 1024

# engine assignment knobs (GPSIMD cannot access PSUM)
K_COPY = os.environ.get("K_COPY", "act")     # a/b psum->sbuf copies: dve|act
K_GRELU = os.environ.get("K_GRELU", "dve")   # gate relu: dve|act|gp
K_MLR = os.environ.get("K_MLR", "act")        # msg leakyrelu: act|dve|gp
K_RECIP = os.environ.get("K_RECIP", "dve")   # reciprocal: act|dve
K_TAIL = os.environ.get("K_TAIL", "dve")      # fea update: gp|dve

_compiled = None


def _build():
    nc = bacc.Bacc("TRN2", target_bir_lowering=False, debug=False,
                   num_devices=NCORES)

    def din(name, shape, dt=F32R):
        return nc.dram_tensor(name, list(shape), dt, kind="ExternalInput").ap()

    ef_d = din("ef_t", (ELEM_EMB + 1, S), BF16)     # row 200 = ones
    sfw_d = din("sfw_t", (SYM_EMB + 2, S), BF16)    # rows: sym, ew, ones
    ew_d = din("ew_t", (1, S), F32)
    embw_d = din("embw", (128, 6 * 32), BF16)
    w1_d = din("w1all", (65, 14 * HID))             # row 64 = bias (B mats)
    w2g_d = din("w2gall", (128, 4 * 2 * 128), BF16)  # 0.99*w2g replicated
    vrow_d = din("vrow", (65, 4))                   # 0.01*W1b@w2g per layer
    powr_d = din("powrow", (1, 4 * 128))
    w2m_d = din("w2mall", (128, 4 * 2 * 64), BF16)
    b1c_d = din("b1call", (128, 4), F32)            # crystal-pool biases
    b2_d = din("b2all", (64, 4), F32)
    identb_d = din("identb", (128, 128), BF16)
    ones1_d = din("ones1", (1, 128))
    onesS_d = din("onesS", (1, S))
    logw1_d = din("logw1", (1, S))
    out_d = nc.dram_tensor("head_aug", [64, CAUG], F32, kind="ExternalOutput").ap()

    with tile.TileContext(nc) as tc:
        with tc.tile_pool(name="persist", bufs=1) as persist:

            fea = persist.tile([65, S], F32R)
            embw_t = persist.tile([128, 6 * 32], BF16)
            w1_t = persist.tile([65, 14 * HID], F32R)
            w2g_t = persist.tile([128, 4 * 2 * 128], BF16)
            vrow_t = persist.tile([65, 4], F32R)
            powr_t = persist.tile([1, 4 * 128], F32R)
            w2m_t = persist.tile([128, 4 * 2 * 64], BF16)
            b1c_t = persist.tile([128, 4], F32)
            b2_t = persist.tile([64, 4], F32)
            identb_t = persist.tile([128, 128], BF16)
            ones1_t = persist.tile([1, 128], F32R)
            logw1 = persist.tile([1, S], F32R)

            for t, d in [(embw_t, embw_d), (w1_t, w1_d), (w2g_t, w2g_d),
                         (vrow_t, vrow_d), (powr_t, powr_d), (w2m_t, w2m_d),
                         (b1c_t, b1c_d), (b2_t, b2_d), (identb_t, identb_d),
                         (ones1_t, ones1_d), (logw1, logw1_d)]:
                nc.sync.dma_start(out=t, in_=d)
            nc.sync.dma_start(out=fea[64:65, :], in_=onesS_d)

            # ---------------- embedding ----------------
            with tc.tile_pool(name="emb_in", bufs=2) as emb_in, \
                 tc.tile_pool(name="emb_ps", bufs=2, space="PSUM") as emb_ps:
                for ch in range(S // EMB_CHUNK):
                    s0 = ch * EMB_CHUNK
                    ef_t = emb_in.tile([128, 2, EMB_CHUNK], BF16, name="ef_t")
                    nc.sync.dma_start(out=ef_t[:, 0, :], in_=ef_d[0:128, s0:s0 + EMB_CHUNK])
                    nc.sync.dma_start(out=ef_t[0:73, 1, :], in_=ef_d[128:201, s0:s0 + EMB_CHUNK])
                    sf_t = emb_in.tile([128, 4, EMB_CHUNK], BF16, name="sf_t")
                    for q in range(3):
                        nc.sync.dma_start(out=sf_t[:, q, :],
                                          in_=sfw_d[q * 128:(q + 1) * 128, s0:s0 + EMB_CHUNK])
                    nc.sync.dma_start(out=sf_t[0:62, 3, :],
                                      in_=sfw_d[384:446, s0:s0 + EMB_CHUNK])
                    for fb in range(EMB_CHUNK // 512):
                        f0 = fb * 512
                        pe1 = emb_ps.tile([32, 512], F32, name="pe1")
                        nc.tensor.matmul(pe1, embw_t[:, 0:32],
                                         ef_t[:, 0, f0:f0 + 512], start=True, stop=False)
                        nc.tensor.matmul(pe1, embw_t[0:73, 32:64],
                                         ef_t[0:73, 1, f0:f0 + 512], start=False, stop=True)
                        pe2 = emb_ps.tile([32, 512], F32, name="pe2")
                        for q in range(3):
                            nc.tensor.matmul(pe2, embw_t[:, 64 + q * 32:96 + q * 32],
                                             sf_t[:, q, f0:f0 + 512],
                                             start=(q == 0), stop=False)
                        nc.tensor.matmul(pe2, embw_t[0:62, 160:192],
                                         sf_t[0:62, 3, f0:f0 + 512], start=False, stop=True)
                        nc.vector.tensor_copy(out=fea[0:32, s0 + f0:s0 + f0 + 512], in_=pe1)
                        nc.vector.tensor_copy(out=fea[32:64, s0 + f0:s0 + f0 + 512], in_=pe2)

            # ---------------- graph layers ----------------
            tail_eng = nc.gpsimd if K_TAIL == "gp" else nc.vector
            for l in range(NG):
                wofs = l * 4 * HID  # w1all: [g_self, g_nbr, m_self, m_nbr]
                with tc.tile_pool(name=f"ab{l}", bufs=3) as abp, \
                     tc.tile_pool(name=f"h{l}", bufs=2) as hp, \
                     tc.tile_pool(name=f"e{l}", bufs=2) as ep, \
                     tc.tile_pool(name=f"sm{l}", bufs=3) as smp, \
                     tc.tile_pool(name=f"psA{l}", bufs=1, space="PSUM") as psA, \
                     tc.tile_pool(name=f"psB{l}", bufs=1, space="PSUM") as psB, \
                     tc.tile_pool(name=f"psL{l}", bufs=2, space="PSUM") as psL, \
                     tc.tile_pool(name=f"psD{l}", bufs=1, space="PSUM") as psD:
                    for hb in range(NHB):
                        s0 = hb * HB
                        # ---- site-level matmuls: A/B for gate and msg ----
                        # psum [128, 2kt, 256]; bias rides w1 row 64 (B mats)
                        pAB = {}
                        for mi, mat in enumerate(["ag", "bg", "am", "bm"]):
                            pool = psA if mi < 2 else psB
                            pA = pool.tile([128, 2, HB], F32, name=f"p_{mat}")
                            kk = 65 if mat in ("bg", "bm") else 64
                            for kt in range(2):
                                nc.tensor.matmul(
                                    pA[:, kt, :],
                                    w1_t[0:kk, wofs + mi * HID + kt * 128:
                                         wofs + mi * HID + (kt + 1) * 128],
                                    fea[0:kk, s0:s0 + HB],
                                    start=True, stop=True, skip_group_check=True)
                            pAB[mat] = pA
                        # beta row: vrow^T @ fea_aug -> [1, 256]
                        pbeta = psD.tile([1, HB], F32, name="pbeta")
                        nc.tensor.matmul(pbeta, vrow_t[:, l:l + 1],
                                         fea[:, s0:s0 + HB], start=True, stop=True,
                                         skip_group_check=True)
                        # ebrow = pow*logw + beta  [1, 256] f32r (DVE: reads PSUM)
                        ebrow = smp.tile([1, HB], F32R, name="ebrow")
                        nc.vector.scalar_tensor_tensor(
                            out=ebrow, in0=logw1[0:1, s0:s0 + HB],
                            scalar=powr_t[0:1, l * 128:l * 128 + 1],
                            in1=pbeta, op0=ALU.mult, op1=ALU.add)

                        # ---- copies psum -> sbuf bf16, kt-interleaved ----
                        ab = {}
                        for mat in ["ag", "bg"]:
                            t = abp.tile([128, HB, 2], BF16, name=f"t_{mat}", tag="ab")
                            src = pAB[mat].rearrange("p k s -> p s k")
                            if K_COPY == "act":
                                nc.scalar.copy(out=t, in_=src)
                            else:
                                nc.vector.tensor_copy(out=t, in_=src)
                            ab[mat] = t
                        for mat in ["am", "bm"]:
                            t = abp.tile([128, 2, HB], F32, name=f"t_{mat}", tag="ab")
                            if mat == "am":
                                nc.scalar.copy(out=t, in_=pAB[mat])
                            else:
                                nc.vector.tensor_copy(out=t, in_=pAB[mat])
                            ab[mat] = t

                        def eview(t, which):
                            v = t.rearrange("p (c x) k -> p c x k", x=ELEMS)
                            if which == "a":  # broadcast along j (axis 3)
                                return v.unsqueeze(3).broadcast_to(
                                    [128, CHB, ELEMS, ELEMS, 2])
                            return v.unsqueeze(2).broadcast_to(
                                [128, CHB, ELEMS, ELEMS, 2])

                        # ---- gate: interleaved add + relu ----
                        hg = hp.tile([128, HB * ELEMS * 2], BF16, name="hg")
                        hgv = hg.rearrange("p (c i j k) -> p c i j k",
                                           i=ELEMS, j=ELEMS, k=2)
                        nc.vector.tensor_tensor(out=hgv, in0=eview(ab["ag"], "a"),
                                                in1=eview(ab["bg"], "b"), op=ALU.add)
                        if K_GRELU == "dve":
                            nc.vector.tensor_scalar_max(out=hg, in0=hg, scalar1=0.0)
                        elif K_GRELU == "gp":
                            nc.gpsimd.tensor_scalar_max(out=hg, in0=hg, scalar1=0.0)
                        else:
                            nc.scalar.activation(out=hg, in_=hg, func=AF.Relu)

                        # ---- logits + exp ----
                        e2 = ep.tile([128, HB * ELEMS], BF16, name="e2")
                        for pb in range(HB * ELEMS // PB):
                            pL = psL.tile([128, PB], F32, name="pL")
                            hgs = hg[:, pb * 2 * PB:(pb + 1) * 2 * PB].rearrange(
                                "p (q k) -> p q k", k=2)
                            for kt in range(2):
                                nc.tensor.matmul(
                                    pL, w2g_t[:, (l * 2 + kt) * 128:(l * 2 + kt + 1) * 128],
                                    hgs[:, :, kt], start=(kt == 0), stop=False)
                            ebv = (ebrow.rearrange("q (c j) -> q c j", j=ELEMS)
                                   [:, pb * (PB // (ELEMS * ELEMS)):(pb + 1) * (PB // (ELEMS * ELEMS)), :]
                                   .unsqueeze(2).broadcast_to(
                                       [1, PB // (ELEMS * ELEMS), ELEMS, ELEMS]))
                            nc.tensor.matmul(pL.rearrange("p (c i j) -> p c i j",
                                                          i=ELEMS, j=ELEMS),
                                             ones1_t, ebv, start=False, stop=True)
                            nc.scalar.activation(
                                out=e2[:, pb * PB:(pb + 1) * PB], in_=pL, func=AF.Exp)

                        e2v = e2.rearrange("p (c i j) -> p c i j", i=ELEMS, j=ELEMS)
                        # ---- denominator: sum_j e, 8 matmuls ----
                        pDM = psD.tile([128, 2 * HB], F32, name="pDM")
                        for j in range(ELEMS):
                            nc.tensor.matmul(pDM[:, 0:HB], identb_t, e2v[:, :, :, j],
                                             start=(j == 0), stop=(j == 7))
                        recip = smp.tile([128, HB], F32, name="recip")
                        if K_RECIP == "act":
                            nc.scalar.activation(out=recip, in_=pDM[:, 0:HB],
                                                 func=AF.Reciprocal)
                        else:
                            nc.vector.reciprocal(out=recip, in_=pDM[:, 0:HB])

                        # ---- msg: per-kt add (DVE/GP), *e, LeakyReLU ----
                        hm = hp.tile([128, 2, HB * ELEMS], BF16, name="hm")

                        def kview(t, kt, which):
                            v = t[:, kt, :].rearrange("p (c x) -> p c x", x=ELEMS)
                            if which == "a":
                                return v.unsqueeze(3).broadcast_to(
                                    [128, CHB, ELEMS, ELEMS])
                            return v.unsqueeze(2).broadcast_to(
                                [128, CHB, ELEMS, ELEMS])

                        hmv = hm.rearrange("p k (c i j) -> p k c i j",
                                           i=ELEMS, j=ELEMS)
                        for kt in range(2):
                            nc.gpsimd.tensor_tensor(
                                out=hmv[:, kt], in0=kview(ab["am"], kt, "a"),
                                in1=kview(ab["bm"], kt, "b"), op=ALU.add)
                        for kt in range(2):
                            nc.vector.tensor_tensor(out=hm[:, kt, :], in0=hm[:, kt, :],
                                                    in1=e2, op=ALU.mult)
                        if K_MLR == "act":
                            nc.scalar.activation(out=hm, in_=hm, func=AF.Prelu,
                                                 alpha=0.01)
                        else:
                            q = hp.tile([128, 2, HB * ELEMS], BF16, name="qlr")
                            nc.vector.tensor_scalar_mul(out=q, in0=hm, scalar1=0.01)
                            nc.vector.tensor_tensor(out=hm, in0=hm, in1=q, op=ALU.max)

                        # ---- W2 contraction with j-summation ----
                        pM = pDM[0:64, HB:2 * HB]
                        i_mm = 0
                        for kt in range(2):
                            for j in range(ELEMS):
                                nc.tensor.matmul(
                                    pM, w2m_t[:, (l * 2 + kt) * 64:(l * 2 + kt + 1) * 64],
                                    hmv[:, kt, :, :, j], start=(i_mm == 0),
                                    stop=(i_mm == 15))
                                i_mm += 1
                        t1 = smp.tile([64, HB], F32, name="t1")
                        nc.vector.tensor_tensor(out=t1, in0=pM, in1=recip[0:64, :],
                                                op=ALU.mult)
                        tail_eng.scalar_tensor_tensor(
                            out=fea[0:64, s0:s0 + HB], in0=t1, scalar=b2_t[:, l:l + 1],
                            in1=fea[0:64, s0:s0 + HB], op0=ALU.add, op1=ALU.add)

            # ---------------- crystal pooling ----------------
            CB = 4096
            with tc.tile_pool(name="cry", bufs=2) as cry, \
                 tc.tile_pool(name="cry1", bufs=1) as cry1, \
                 tc.tile_pool(name="cry_ps", bufs=2, space="PSUM") as cry_ps, \
                 tc.tile_pool(name="cry_psD", bufs=1, space="PSUM") as cry_psD:
                logw_cry = logw1
                for cb in range(S // CB):
                    s0 = cb * CB
                    e_c = cry.tile([128, CB], BF16, name="e_c", bufs=1)
                    hc = cry.tile([128, 2, CB], BF16, name="hc", bufs=1)
                    for fb in range(CB // 512):
                        f0 = s0 + fb * 512
                        hg2 = cry.tile([128, 2, 512], BF16, name="chg")
                        for kt in range(2):
                            pH = cry_ps.tile([128, 512], F32, name="cpH")
                            nc.tensor.matmul(
                                pH, w1_t[0:64, 12 * HID + kt * 128:12 * HID + (kt + 1) * 128],
                                fea[0:64, f0:f0 + 512], start=True, stop=True)
                            nc.scalar.activation(out=hg2[:, kt, :], in_=pH, func=AF.Prelu,
                                                 bias=b1c_t[:, kt:kt + 1], alpha=0.01)
                        pL = cry_ps.tile([128, 512], F32, name="cpL")
                        for kt in range(2):
                            nc.tensor.matmul(pL, w2g_t[:, (6 + kt) * 128:(7 + kt) * 128],
                                             hg2[:, kt, :], start=(kt == 0), stop=False)
                        nc.tensor.matmul(pL, powr_t[:, 3 * 128:4 * 128],
                                         logw_cry[:, f0:f0 + 512], start=False, stop=True)
                        nc.scalar.activation(out=e_c[:, fb * 512:(fb + 1) * 512], in_=pL,
                                             func=AF.Exp)
                        for kt in range(2):
                            pH2 = cry_ps.tile([128, 512], F32, name="cpH2")
                            nc.tensor.matmul(
                                pH2, w1_t[0:64, 13 * HID + kt * 128:13 * HID + (kt + 1) * 128],
                                fea[0:64, f0:f0 + 512], start=True, stop=True)
                            nc.scalar.activation(out=hc[:, kt, fb * 512:(fb + 1) * 512],
                                                 in_=pH2, func=AF.Prelu,
                                                 bias=b1c_t[:, 2 + kt:3 + kt], alpha=0.01)
                    for kt in range(2):
                        nc.vector.tensor_tensor(out=hc[:, kt, :], in0=hc[:, kt, :],
                                                in1=e_c, op=ALU.mult)
                    pD = cry_psD.tile([128, CB // ELEMS], F32, name="cpD")
                    ecv = e_c.rearrange("p (s j) -> p s j", j=ELEMS)
                    for j in range(ELEMS):
                        nc.tensor.matmul(pD, identb_t, ecv[:, :, j],
                                         start=(j == 0), stop=(j == 7))
                    crecip = cry.tile([128, CB // ELEMS], F32, name="crecip")
                    nc.vector.reciprocal(out=crecip, in_=pD)
                    pM = cry_psD.tile([64, CB // ELEMS], F32, name="cpM")
                    hcv = hc.rearrange("p k (s j) -> p k s j", j=ELEMS)
                    i_mm = 0
                    for kt in range(2):
                        for j in range(ELEMS):
                            nc.tensor.matmul(pM, w2m_t[:, (6 + kt) * 64:(7 + kt) * 64],
                                             hcv[:, kt, :, j], start=(i_mm == 0),
                                             stop=(i_mm == 15))
                            i_mm += 1
                    t1 = cry.tile([64, CB // ELEMS], F32, name="ct1")
                    nc.vector.tensor_tensor(out=t1, in0=pM, in1=crecip[0:64, :],
                                            op=ALU.mult)
                    # aug mean over groups of 4 crystals, then *1/4 + b2
                    havg = cry.tile([64, CB // ELEMS // AUG], F32, name="havg")
                    nc.vector.tensor_reduce(
                        out=havg.unsqueeze(2),
                        in_=t1.rearrange("p (g a) -> p g a", a=AUG),
                        axis=mybir.AxisListType.X, op=ALU.add)
                    nc.vector.tensor_scalar(out=havg, in0=havg, scalar1=1.0 / AUG,
                                            scalar2=b2_t[:, 3:4], op0=ALU.mult,
                                            op1=ALU.add)
                    nofs = CB // ELEMS // AUG
                    nc.sync.dma_start(out=out_d[:, cb * nofs:(cb + 1) * nofs], in_=havg)

    nc.compile()
    return nc


def _prep(inputs):
    import ml_dtypes
    ew = np.asarray(inputs["elem_weights"], np.float32)
    ef = np.asarray(inputs["elem_fea"], np.float32)
    sf = np.asarray(inputs["sym_fea"], np.float32)

    embw = np.zeros((128, 6 * 32), np.float32)
    embw[:, 0:32] = inputs["elem_W"][0:128]
    embw[0:72, 32:64] = inputs["elem_W"][128:200]
    embw[72, 32:64] = inputs["elem_b"]          # ones-row bias for elem
    symW = np.asarray(inputs["sym_W"], np.float32)
    for q in range(3):
        embw[:, 64 + q * 32:96 + q * 32] = symW[q * 128:(q + 1) * 128]
    embw[0:61, 160:192] = symW[384:445]         # rows 384..444 (sym + ew row)
    embw[61, 160:192] = inputs["sym_b"]         # ones-row bias for sym

    # w1all [65, 14*HID]: per layer 4 mats [g_self, g_nbr, m_self, m_nbr];
    # bias row 64 on the nbr (B) mats
    w1 = np.zeros((65, 14 * HID), np.float32)
    for l in range(NG):
        w1[0:64, (l * 4 + 0) * HID:(l * 4 + 1) * HID] = inputs["g_gate_W1"][l][0:64]
        w1[0:64, (l * 4 + 1) * HID:(l * 4 + 2) * HID] = inputs["g_gate_W1"][l][64:128]
        w1[64, (l * 4 + 1) * HID:(l * 4 + 2) * HID] = inputs["g_gate_b1"][l]
        w1[0:64, (l * 4 + 2) * HID:(l * 4 + 3) * HID] = inputs["g_msg_W1"][l][0:64]
        w1[0:64, (l * 4 + 3) * HID:(l * 4 + 4) * HID] = inputs["g_msg_W1"][l][64:128]
        w1[64, (l * 4 + 3) * HID:(l * 4 + 4) * HID] = inputs["g_msg_b1"][l]
    w1[0:64, 12 * HID:13 * HID] = inputs["c_gate_W1"]
    w1[0:64, 13 * HID:14 * HID] = inputs["c_msg_W1"]

    b1c = np.zeros((128, 4), np.float32)
    for kt in range(2):
        b1c[:, kt] = inputs["c_gate_b1"][kt * 128:(kt + 1) * 128]
        b1c[:, 2 + kt] = inputs["c_msg_b1"][kt * 128:(kt + 1) * 128]

    w2g = np.zeros((128, 4 * 2 * 128), np.float32)
    vrow = np.zeros((65, 4), np.float32)
    powr = np.zeros((1, 4 * 128), np.float32)
    w2m = np.zeros((128, 4 * 2 * 64), np.float32)
    b2 = np.zeros((64, 4), np.float32)
    for l in range(NG):
        gw2 = np.asarray(inputs["g_gate_W2"][l], np.float32)
        for kt in range(2):
            w2g[:, (l * 2 + kt) * 128:(l * 2 + kt + 1) * 128] = \
                np.repeat(0.99 * gw2[kt * 128:(kt + 1) * 128], 128, axis=1)
            w2m[:, (l * 2 + kt) * 64:(l * 2 + kt + 1) * 64] = \
                inputs["g_msg_W2"][l][kt * 128:(kt + 1) * 128]
        # vrow: 0.01 * (W1_nbr_aug @ w2g)  [65, 1] per layer
        w1b_aug = np.zeros((65, 256), np.float32)
        w1b_aug[0:64] = inputs["g_gate_W1"][l][64:128]
        w1b_aug[64] = inputs["g_gate_b1"][l]
        vrow[:, l] = 0.01 * (w1b_aug @ gw2)[:, 0]
        powr[0, l * 128:(l + 1) * 128] = float(inputs["g_pow"][l])
        b2[:, l] = inputs["g_msg_b2"][l]
    cw2 = np.asarray(inputs["c_gate_W2"], np.float32)
    for kt in range(2):
        w2g[:, (6 + kt) * 128:(7 + kt) * 128] = np.repeat(cw2[kt * 128:(kt + 1) * 128],
                                                          128, axis=1)
        w2m[:, (6 + kt) * 64:(7 + kt) * 64] = inputs["c_msg_W2"][kt * 128:(kt + 1) * 128]
    powr[0, 3 * 128:4 * 128] = float(inputs["c_pow"])
    b2[:, 3] = inputs["c_msg_b2"]

    logw = np.log(ew[:, 0])  # [N]

    common = dict(embw=embw.astype(ml_dtypes.bfloat16),
                  w1all=w1, w2gall=w2g.astype(ml_dtypes.bfloat16),
                  vrow=vrow, powrow=powr,
                  w2mall=w2m.astype(ml_dtypes.bfloat16),
                  b1call=b1c, b2all=b2,
                  identb=np.eye(128, dtype=ml_dtypes.bfloat16),
                  ones1=np.ones((1, 128), np.float32),
                  onesS=np.ones((1, S), np.float32))

    in_maps = []
    for k in range(NCORES):
        sl = slice(k * S, (k + 1) * S)
        m = dict(common)
        ef_aug = np.concatenate([ef[sl], np.ones((S, 1), np.float32)], axis=1)
        sf_aug = np.concatenate([sf[sl], ew[sl], np.ones((S, 1), np.float32)], axis=1)
        m["ef_t"] = np.ascontiguousarray(ef_aug.T).astype(ml_dtypes.bfloat16)
        m["sfw_t"] = np.ascontiguousarray(sf_aug.T).astype(ml_dtypes.bfloat16)
        m["ew_t"] = np.ascontiguousarray(ew[sl].T)
        m["logw1"] = np.ascontiguousarray(logw[sl].reshape(1, S))
        in_maps.append(m)
    return in_maps


def kernel(**inputs):
    global _compiled
    if _compiled is None:
        _compiled = _build()
    in_maps = _prep(inputs)
    res = run_bass_kernel_spmd(_compiled, in_maps, core_ids=list(range(NCORES)))
    outs = [r["head_aug"].T for r in res.results]
    return np.ascontiguousarray(np.concatenate(outs, axis=0), dtype=np.float32)
